# revision 1
# baseline (speedup 1.0000x reference)
"""Trainium2 Bass kernel for a dense transformer block (attention + GeGLU-mish
FFN) on x:[2,2048,768], distributed over 8 NeuronCores.

Sharding: core i handles batch i//4, query-block i%4 (512 rows). K/V for the
full 2048-token batch are computed redundantly per core (no collectives).
All activations are kept feature-major (D on partitions) so every matmul's
contraction dim lands on partitions with no on-device transposes. The host
rotates each core's sequence so its own query block is always block 0
(attention is permutation-invariant over keys), letting all cores run one
identical SPMD program.
"""
import sys

sys.path.insert(0, "/opt/trn_rl_repo")

import numpy as np
import ml_dtypes

import bass_rust
import concourse.bass as bass
import concourse.mybir as mybir
import concourse.tile as tile
from concourse.bass_utils import run_bass_kernel_spmd

AF = mybir.ActivationFunctionType
ALU = mybir.AluOpType
BF16 = mybir.dt.bfloat16
F32 = mybir.dt.float32
F32R = mybir.dt.float32r

MISH_FUNC = None  # set to an AF member to override (e.g. sim lacks Mish)

DIM = 768
NH = 12
HD = 64
HIDDEN = 3072
S = 2048
QB = 512          # query rows per core
EPS = 1e-5
NCK = DIM // 128  # 6 chunks of the model dim

# ---------------------------------------------------------------------------
# Workaround for a walrus codegen limit: an instruction may carry at most one
# sync-wait command, but TileContext's exit drain accumulates one wait per
# logical proc. Split the waits onto chained SP-engine NOPs before the drain.
# ---------------------------------------------------------------------------


CTRL_OPCODES = ("Drain", "NoOp", "Nop", "EventSemOp", "SemOp", "Branch")


def _wait_limit(opcode):
    return 1


def _split_waits(nc):
    """Hoist excess per-instruction sem waits onto same-engine NOPs (this
    walrus build accepts at most 1 wait on CTRL ops / 2 on compute ops)."""
    for f in nc.m.functions:
        for bb in f.blocks:
            snapshot = list(bb.instructions)
            new = []
            for inst in snapshot:
                si = inst.sync_info
                waits = list(si.on_wait) if si and si.on_wait else []
                limit = _wait_limit(inst.opcode)
                if len(waits) > limit:
                    si.on_wait = waits[:limit]
                    eng = nc.engines[inst.engine]
                    for w in waits[limit:]:
                        nop = eng.nop()
                        popped = nc.cur_bb.bb.instructions.pop()
                        assert popped is nop.ins
                        nop.ins.sync_info = bass_rust.SyncInfo(
                            on_wait=[w], on_update=[])
                        new.append(nop.ins)
                new.append(inst)
            bb.instructions[:] = new


# ---------------------------------------------------------------------------
# Device program
# ---------------------------------------------------------------------------

def _bcast_ap(dram_tile, offset_elems, n_part, inner):
    """AP replicating DRAM data across n_part partitions (step-0 leading dim)."""
    return bass.AP(
        tensor=dram_tile.tensor,
        offset=dram_tile.offset + offset_elems,
        ap=[[0, n_part]] + inner,
    )


def build_nc(repeat=1):
    nc = bass.Bass()
    xT_d = nc.dram_tensor("xT", [DIM, QB], F32, kind="ExternalInput")
    wqkv_d = nc.dram_tensor("wqkv", [DIM, 3 * DIM], BF16, kind="ExternalInput")
    wout_d = nc.dram_tensor("wout", [DIM, DIM], BF16, kind="ExternalInput")
    # w1 pre-tiled on host: [48, 128, 6, 128] (col-chunk, p, d-chunk, col)
    w1t_d = nc.dram_tensor("w1t", [48, 128, NCK, 128], BF16, kind="ExternalInput")
    w2_d = nc.dram_tensor("w2", [HIDDEN, DIM], BF16, kind="ExternalInput")
    yT_d = nc.dram_tensor("yT", [DIM, QB], F32, kind="ExternalOutput")

    with tile.TileContext(nc) as tc:
        for _ in range(repeat):
            _body(nc, tc, xT_d, wqkv_d, wout_d, w1t_d, w2_d, yT_d)
    _split_waits(nc)
    return nc


def _body(nc, tc, xT_d, wqkv_d, wout_d, w1t_d, w2_d, yT_d):
    from contextlib import ExitStack

    ctx = ExitStack()
    with ctx:
        singles = ctx.enter_context(tc.tile_pool(name="singles", bufs=1))
        dram = ctx.enter_context(tc.tile_pool(name="dram", bufs=1, space="DRAM"))

        ones_f = singles.tile([128, 1], F32)
        nc.vector.memset(ones_f[:], 1.0)
        ones = singles.tile([128, 1], F32R)
        nc.vector.tensor_copy(ones[:], ones_f[:])
        eps1 = singles.tile([1, 1], F32)
        nc.vector.memset(eps1[:], EPS)
        m_all = singles.tile([1, 128], F32)
        nc.vector.memset(m_all[:], 1.0)
        m_lo = singles.tile([1, 128], F32)
        nc.vector.memset(m_lo[:], 0.0)
        nc.vector.memset(m_lo[0:1, 0:64], 1.0)
        m_hi = singles.tile([1, 128], F32)
        nc.vector.memset(m_hi[:], 0.0)
        nc.vector.memset(m_hi[0:1, 64:128], 1.0)

        # persistent activations (live across phases)
        x_own = [singles.tile([128, QB], F32, name=f"xo_{c}") for c in range(NCK)]
        for c in range(NCK):
            eng = nc.gpsimd if c % 2 else nc.sync
            eng.dma_start(x_own[c][:], xT_d[c * 128:(c + 1) * 128, :])
        x1 = [singles.tile([128, QB], F32, name=f"x1_{c}") for c in range(NCK)]
        xh1 = [singles.tile([128, QB], BF16, name=f"xh1_{c}") for c in range(NCK)]
        wout_sb = [singles.tile([128, DIM], BF16, name=f"wout_{c}")
                   for c in range(NCK)]

        with tc.tile_pool(name="p23", bufs=1) as p23, \
             tc.tile_pool(name="cc", bufs=1, space="DRAM") as ccp:
            kT = [p23.tile([128, S], BF16, name=f"kT_{c}") for c in range(NCK)]
            qT = [p23.tile([128, QB], BF16, name=f"qT_{c}") for c in range(NCK)]
            v_t = [p23.tile([128, NH, HD + 1], BF16, name=f"v_{t}")
                   for t in range(16)]
            wv_sb = [p23.tile([128, QB], BF16, name=f"wv_{c}") for c in range(NCK)]
            KSZ = NCK * 128 * QB
            VSZ = 4 * 128 * NH * (HD + 1)
            kv_in = ccp.tile([KSZ + VSZ], BF16)
            kv_out = ccp.tile([4, KSZ + VSZ], BF16)

            with tc.tile_pool(name="ph12", bufs=1) as ph12:
                wqkv_sb = []
                for c in range(NCK):
                    t = ph12.tile([128, 3 * DIM], BF16, name=f"wqkv_{c}")
                    nc.sync.dma_start(t[:], wqkv_d[c * 128:(c + 1) * 128, :])
                    wqkv_sb.append(t)
                for c in range(NCK):
                    nc.sync.dma_start(wout_sb[c][:],
                                      wout_d[c * 128:(c + 1) * 128, :])

                # ---------------- phase 1: norm1 (own block only) ---------
                with tc.tile_pool(name="n1", bufs=3) as n1, \
                     tc.tile_pool(name="n1ps", bufs=1, space="PSUM") as n1ps, \
                     tc.tile_pool(name="n1bc", bufs=1, space="PSUM") as n1bc:
                    ss_ps = n1ps.tile([1, QB], F32, name="ss")
                    xh0 = [ph12.tile([128, QB], BF16, name=f"xh_{c}")
                           for c in range(NCK)]
                    for c in range(NCK):
                        xt = x_own[c][:]
                        sq = n1.tile([128, QB], F32R, name="sq")
                        eng = nc.gpsimd if c < 4 else nc.vector
                        eng.tensor_tensor(sq[:], xt, xt, ALU.mult)
                        nc.tensor.matmul(ss_ps[:], ones[:], sq[:],
                                         start=(c == 0), stop=(c == NCK - 1))
                    rstd0 = n1.tile([1, QB], F32, name="rstd0")
                    nc.scalar.activation(out=rstd0[:], in_=ss_ps[:], func=AF.Sqrt,
                                         bias=eps1[:], scale=1.0 / DIM)
                    rstd = n1.tile([1, QB], F32, name="rstd")
                    nc.vector.reciprocal(rstd[:], rstd0[:])
                    rbc = n1bc.tile([128, QB], F32, name="rbc")
                    nc.tensor.matmul(rbc[:], m_all[:], rstd[:],
                                     start=True, stop=True)
                    for c in range(NCK):
                        nc.vector.tensor_tensor(
                            xh0[c][:], x_own[c][:], rbc[:], ALU.mult)

                # ---------------- phase 2: qkv (own block) + all-gather ------
                with tc.tile_pool(name="qkps", bufs=2, space="PSUM") as qkps, \
                     tc.tile_pool(name="vps", bufs=2, space="PSUM") as vps, \
                     tc.tile_pool(name="kst", bufs=1) as kst:
                    # k (own block) -> stage -> dram
                    k_st = [kst.tile([128, QB], BF16, name=f"kst_{c}")
                            for c in range(NCK)]
                    for oc in range(NCK):
                        ps = qkps.tile([128, QB], F32, name="qk")
                        for c in range(NCK):
                            nc.tensor.matmul(
                                ps[:],
                                wqkv_sb[c][:, DIM + oc * 128:DIM + (oc + 1) * 128],
                                xh0[c][:],
                                start=(c == 0), stop=(c == NCK - 1))
                        nc.scalar.copy(out=k_st[oc][:], in_=ps[:])
                        nc.sync.dma_start(
                            kv_in[oc * 128 * QB:(oc + 1) * 128 * QB]
                            .rearrange("(p q) -> p q", p=128),
                            k_st[oc][:])
                    # v (own block, token-major + ones col) -> stage -> dram
                    v_st = [kst.tile([128, NH, HD + 1], BF16, name=f"vst_{t}")
                            for t in range(4)]
                    for t in range(4):
                        ps = vps.tile([128, DIM], F32, name="v")
                        for off, width in ((0, 512), (512, 256)):
                            for c in range(NCK):
                                nc.tensor.matmul(
                                    ps[:, off:off + width],
                                    xh0[c][:, t * 128:(t + 1) * 128],
                                    wqkv_sb[c][:, 2 * DIM + off:
                                               2 * DIM + off + width],
                                    start=(c == 0), stop=(c == NCK - 1))
                        nc.vector.tensor_copy(
                            v_st[t][:, :, 0:HD],
                            ps.rearrange("p (h d) -> p h d", h=NH))
                        nc.vector.memset(v_st[t][:, :, HD:HD + 1], 1.0)
                        vw = NH * (HD + 1)
                        nc.sync.dma_start(
                            kv_in[KSZ + t * 128 * vw:KSZ + (t + 1) * 128 * vw]
                            .rearrange("(p w) -> p w", p=128),
                            v_st[t].rearrange("p h d -> p (h d)"))
                    nc.gpsimd.collective_compute(
                        "AllGather", ALU.bypass,
                        replica_groups=[[0, 1, 2, 3], [4, 5, 6, 7]],
                        ins=[kv_in.opt()], outs=[kv_out.opt()])
                    # q (own block)
                    for oc in range(NCK):
                        ps = qkps.tile([128, QB], F32, name="qk")
                        for c in range(NCK):
                            nc.tensor.matmul(
                                ps[:], wqkv_sb[c][:, oc * 128:(oc + 1) * 128],
                                xh0[c][:],
                                start=(c == 0), stop=(c == NCK - 1))
                        nc.scalar.copy(out=qT[oc][:], in_=ps[:])
                    # load gathered k/v back
                    kvap = kv_out[:, :]
                    TOT = KSZ + VSZ
                    for c in range(NCK):
                        srcap = bass.AP(
                            tensor=kvap.tensor,
                            offset=kvap.offset + c * 128 * QB,
                            ap=[[QB, 128], [TOT, 4], [1, QB]])
                        nc.sync.dma_start(kT[c][:].rearrange("p (b q) -> p b q", b=4),
                                          srcap)
                    vw = NH * (HD + 1)
                    for t in range(16):
                        srcap = bass.AP(
                            tensor=kvap.tensor,
                            offset=kvap.offset + (t // 4) * TOT + KSZ
                                   + (t % 4) * 128 * vw,
                            ap=[[vw, 128], [1, vw]])
                        nc.sync.dma_start(
                            v_t[t][:].rearrange("p h d -> p (h d)"), srcap)

        # ---------------- phase 3: attention ----------------------------
            with tc.tile_pool(name="att_ps", bufs=2, space="PSUM") as ps_p, \
                 tc.tile_pool(name="att_wv", bufs=2, space="PSUM") as wv_p, \
                 tc.tile_pool(name="att_rec", bufs=1, space="PSUM") as rec_p, \
                 tc.tile_pool(name="att_pT", bufs=3) as pT_p, \
                 tc.tile_pool(name="att_den", bufs=4) as den_p, \
                 tc.tile_pool(name="att_den1", bufs=1) as den1_p:
                denA = den1_p.tile([1, NCK * QB], F32, name="denA")
                denB = den1_p.tile([1, NCK * QB], F32, name="denB")
                for c in range(NCK):
                    hA, hB = 2 * c, 2 * c + 1
                    wvA = wv_p.tile([128, QB], F32, name="wvps")
                    wvB = wv_p.tile([128, QB], F32, name="wvps")
                    for kt in range(16):
                        kslc = kT[c][:, kt * 128:(kt + 1) * 128]
                        ps = ps_p.tile([128, 1024], F32, name="sAB")
                        nc.tensor.matmul(
                            ps[:, 0:QB], kslc[0:64, :],
                            qT[c][0:64, :], start=True, stop=True)
                        nc.tensor.matmul(
                            ps[:, QB:2 * QB], kslc[64:128, :],
                            qT[c][64:128, :], start=True, stop=True,
                            tile_position=(64, 0))
                        pT = pT_p.tile([128, 1024], BF16, name="pT")
                        nc.scalar.activation(out=pT[:], in_=ps[:], func=AF.Exp)
                        nc.tensor.matmul(
                            wvA[0:HD + 1, :], v_t[kt][:, hA, :],
                            pT[:, 0:QB], start=(kt == 0), stop=(kt == 15))
                        nc.tensor.matmul(
                            wvB[0:HD + 1, :], v_t[kt][:, hB, :],
                            pT[:, QB:2 * QB], start=(kt == 0), stop=(kt == 15))
                    # reciprocal denominators -> denA/denB cols for chunk c
                    for den, wv in ((denA, wvA), (denB, wvB)):
                        dent = den_p.tile([1, QB], F32, name="den")
                        nc.vector.tensor_copy(dent[:], wv[HD:HD + 1, :])
                        nc.vector.reciprocal(
                            den[0:1, c * QB:(c + 1) * QB], dent[:])
                    # stash unnormalized wv, then normalize via K=1 bcast matmul
                    nc.vector.tensor_copy(wv_sb[c][0:64, :], wvA[0:HD, :])
                    nc.vector.tensor_copy(wv_sb[c][64:128, :], wvB[0:HD, :])
                    rec = rec_p.tile([128, QB], F32, name="rec")
                    nc.tensor.matmul(rec[:], m_lo[:],
                                     denA[0:1, c * QB:(c + 1) * QB],
                                     start=True, stop=False)
                    nc.tensor.matmul(rec[:], m_hi[:],
                                     denB[0:1, c * QB:(c + 1) * QB],
                                     start=False, stop=True)
                    nc.vector.tensor_tensor(
                        wv_sb[c][:], wv_sb[c][:], rec[:], ALU.mult)

        # ---------------- phases 4+5 share weight pools -----------------
        w1s = ctx.enter_context(tc.tile_pool(name="w1s", bufs=3))
        w2s = ctx.enter_context(tc.tile_pool(name="w2s", bufs=1))
        ffu = ctx.enter_context(tc.tile_pool(name="ffu", bufs=1))
        u_sb = [ffu.tile([128, QB], BF16, name=f"u_{j}") for j in range(24)]
        w2_sb = [w2s.tile([128, DIM], BF16, name=f"w2_{j}") for j in range(24)]
        pre_w1 = {}
        for j in range(2):
            w1a = w1s.tile([128, NCK, 128], BF16, name="w1a")
            nc.sync.dma_start(w1a[:], w1t_d[j])
            w1b = w1s.tile([128, NCK, 128], BF16, name="w1b")
            nc.sync.dma_start(w1b[:], w1t_d[24 + j])
            pre_w1[j] = (w1a, w1b)
            nc.sync.dma_start(w2_sb[j][:], w2_d[j * 128:(j + 1) * 128, :])

        # ---------------- phase 4: out-proj + residual + norm2 ----------
        with tc.tile_pool(name="op_ps", bufs=3, space="PSUM") as op_ps, \
             tc.tile_pool(name="n2ps", bufs=1, space="PSUM") as n2ps, \
             tc.tile_pool(name="n2", bufs=3) as n2:
            for oc in range(NCK):
                ps = op_ps.tile([128, QB], F32, name="op")
                for c in range(NCK):
                    nc.tensor.matmul(
                        ps[:], wout_sb[c][:, oc * 128:(oc + 1) * 128],
                        wv_sb[c][:], start=(c == 0), stop=(c == NCK - 1))
                nc.vector.tensor_tensor(x1[oc][:], ps[:], x_own[oc][:], ALU.add)
            # norm2
            ss_ps = n2ps.tile([1, QB], F32, name="ss2")
            for c in range(NCK):
                sq = n2.tile([128, QB], F32R, name="sq2")
                eng = nc.gpsimd if c < 4 else nc.vector
                eng.tensor_tensor(sq[:], x1[c][:], x1[c][:], ALU.mult)
                nc.tensor.matmul(ss_ps[:], ones[:], sq[:],
                                 start=(c == 0), stop=(c == NCK - 1))
            rstd0 = n2.tile([1, QB], F32, name="rstd20")
            nc.scalar.activation(out=rstd0[:], in_=ss_ps[:], func=AF.Sqrt,
                                 bias=eps1[:], scale=1.0 / DIM)
            rstd = n2.tile([1, QB], F32, name="rstd2")
            nc.vector.reciprocal(rstd[:], rstd0[:])
            rbc = n2ps.tile([128, QB], F32, name="rbc2")
            nc.tensor.matmul(rbc[:], m_all[:], rstd[:], start=True, stop=True)
            for c in range(NCK):
                nc.vector.tensor_tensor(xh1[c][:], x1[c][:], rbc[:], ALU.mult)

        # ---------------- phase 5: FFN ----------------------------------
        with tc.tile_pool(name="ffn", bufs=3) as ffn:
            for half in range(2):
                with tc.tile_pool(name=f"y_ps{half}", bufs=1,
                                  space="PSUM") as y_ps, \
                     tc.tile_pool(name=f"f_psh{half}", bufs=3,
                                  space="PSUM") as f_psh, \
                     tc.tile_pool(name=f"f_ps{half}", bufs=2,
                                  space="PSUM") as f_ps:
                    yps = y_ps.tile([128, 3 * QB], F32, name="y")
                    for j in range(24):
                        if half == 0:
                            if j in pre_w1:
                                w1a, w1b = pre_w1[j]
                            else:
                                w1a = w1s.tile([128, NCK, 128], BF16, name="w1a")
                                nc.sync.dma_start(w1a[:], w1t_d[j])
                                w1b = w1s.tile([128, NCK, 128], BF16, name="w1b")
                                nc.sync.dma_start(w1b[:], w1t_d[24 + j])
                            psg = f_ps.tile([128, QB], F32, name="psg")
                            psh = f_psh.tile([128, QB], F32, name="psh")
                            for c in range(NCK):
                                nc.tensor.matmul(psg[:], w1b[:, c, :], xh1[c][:],
                                                 start=(c == 0),
                                                 stop=(c == NCK - 1))
                            for c in range(NCK):
                                nc.tensor.matmul(psh[:], w1a[:, c, :], xh1[c][:],
                                                 start=(c == 0),
                                                 stop=(c == NCK - 1))
                            # free psg quickly via g copy; free psh via exp+G
                            g = ffn.tile([128, QB], F32, name="gate")
                            nc.vector.tensor_copy(g[:], psg[:])
                            me = ffn.tile([128, QB], F32, name="mish_e")
                            nc.scalar.activation(out=me[:], in_=psh[:],
                                                 func=AF.Exp)
                            G = ffn.tile([128, QB], BF16, name="mish_G")
                            nc.vector.tensor_tensor(G[:], psh[:], g[:], ALU.mult)
                            # mish(x)*gate = G * (1 - 2/((1+e^x)^2+1))
                            w2q = ffn.tile([128, QB], F32, name="mish_w2")
                            nc.scalar.activation(out=w2q[:], in_=me[:],
                                                 func=AF.Square, bias=1.0)
                            d = ffn.tile([128, QB], F32, name="mish_d")
                            nc.scalar.activation(out=d[:], in_=w2q[:],
                                                 func=AF.Identity, bias=1.0)
                            r = ffn.tile([128, QB], F32, name="mish_r")
                            nc.vector.reciprocal(r[:], d[:])
                            t = ffn.tile([128, QB], F32, name="mish_t")
                            nc.scalar.activation(out=t[:], in_=r[:],
                                                 func=AF.Identity,
                                                 scale=-2.0, bias=1.0)
                            nc.vector.tensor_tensor(u_sb[j][:], G[:], t[:],
                                                    ALU.mult)
                            if j >= 2:
                                nc.sync.dma_start(
                                    w2_sb[j][:],
                                    w2_d[j * 128:(j + 1) * 128, :])
                        for o3 in range(3):
                            oc = half * 3 + o3
                            nc.tensor.matmul(
                                yps[:, o3 * QB:(o3 + 1) * QB],
                                w2_sb[j][:, oc * 128:(oc + 1) * 128], u_sb[j][:],
                                start=(j == 0), stop=(j == 23),
                                skip_group_check=True)
                    for o3 in range(3):
                        oc = half * 3 + o3
                        yout = ffn.tile([128, QB], F32, name="yout")
                        nc.vector.tensor_tensor(
                            yout[:], yps[:, o3 * QB:(o3 + 1) * QB],
                            x1[oc][:], ALU.add)
                        nc.sync.dma_start(
                            yT_d[oc * 128:(oc + 1) * 128, :], yout[:])


# ---------------------------------------------------------------------------
# Host wrapper
# ---------------------------------------------------------------------------

_NC_CACHE = {}


def _get_nc():
    if "nc" not in _NC_CACHE:
        _NC_CACHE["nc"] = build_nc()
    return _NC_CACHE["nc"]


def _prep_inputs(x, w_qkv, w_out, w1, w2, g_attn, g_ff):
    bf16 = ml_dtypes.bfloat16
    scale = 1.0 / np.sqrt(HD)
    wqkv_f = (g_attn[:, None] * w_qkv).astype(np.float32).copy()
    wqkv_f[:, : 2 * DIM] *= scale  # fold attention scale into q AND k weights
    wqkv_b = np.ascontiguousarray(wqkv_f.astype(bf16))
    wout_b = np.ascontiguousarray(w_out.astype(bf16))
    w1_f = (g_ff[:, None] * w1).astype(np.float32)
    # pre-tile w1: [768, 6144] -> [48, 128, 6, 128] (col-chunk, p, d-chunk, col)
    w1t = np.ascontiguousarray(
        w1_f.reshape(NCK, 128, 48, 128).transpose(2, 1, 0, 3).astype(bf16))
    w2_b = np.ascontiguousarray(w2.astype(bf16))

    in_maps = []
    for core in range(8):
        b, qb = core // 4, core % 4
        xT = np.ascontiguousarray(
            x[b][qb * QB:(qb + 1) * QB].T.astype(np.float32))
        in_maps.append({
            "xT": xT,
            "wqkv": wqkv_b,
            "wout": wout_b,
            "w1t": w1t,
            "w2": w2_b,
        })
    return in_maps


def run(x, w_qkv, w_out, w1, w2, g_attn, g_ff, trace=False, **kw):
    nc = _get_nc()
    in_maps = _prep_inputs(x, w_qkv, w_out, w1, w2, g_attn, g_ff)
    res = run_bass_kernel_spmd(
        nc, in_maps, core_ids=list(range(8)), trace=trace, **kw)
    B = x.shape[0]
    y = np.zeros((B, S, DIM), dtype=np.float32)
    for core in range(8):
        b, qb = core // 4, core % 4
        yT = res.results[core]["yT"]  # [768, 512]
        y[b, qb * QB:(qb + 1) * QB, :] = np.asarray(yT).T
    return y, res


def kernel(x, w_qkv, w_out, w1, w2, g_attn, g_ff):
    y, _ = run(np.asarray(x, np.float32), np.asarray(w_qkv, np.float32),
               np.asarray(w_out, np.float32), np.asarray(w1, np.float32),
               np.asarray(w2, np.float32), np.asarray(g_attn, np.float32),
               np.asarray(g_ff, np.float32))
    return y

